# revision 21
# baseline (speedup 1.0000x reference)
"""AtomPosGNN distributed Trainium2 kernel (8 NeuronCores) — v2.

Reference computation (N=8192 nodes, H=128 features, L=4 layers):
    feat = concat(atom_pos, atom_emb)            # [N, 128]
    deg = dist_adj.sum(-1); isd = rsqrt(deg)
    for l in range(4):
        h = (feat * isd[:, None]) @ Ws[l]
        h = dist_adj @ h
        feat = softplus(h * isd[:, None] + bs[l])

Strategy (row shard, P=1024 rows per core):
  - Host ships adj^T for the local row-block PRE-TILED into the exact
    SBUF layout [128p, 64 slot, 1024c] (slot = (khi, r, klo) AG order)
    and cast to fp8e4 (numerically free for adj: verified 1.08e-3 final
    rel err, same as bf16). 8MB/core, one pass, 64KB/partition lines.
  - deg via PE matmul with an all-ones [128,128] fp8 stationary: output
    is deg replicated across partitions (no DRAM broadcast bounce).
    Overlaps the adj load.
  - Per layer: g = (feat*isd)@W in two node-halves (khi); each half is
    staged and AllGathered independently (2 collectives/layer) so the
    gather pipelines behind the big matmul of the previous half/chunk.
  - Big matmul y^T[h,c] = sum_n g[n,h] adjT[n,c]: g tiles stationary
    (bf16), adj^T streams (fp8), 2 column chunks of 512 for
    epilogue/AG overlap; epilogue = DVE isd-mult + ACT Softplus.
  - adj is read from HBM exactly once; layers run entirely from SBUF.
"""

import os
import sys

for _p in ("/opt/trn_rl_repo",):
    if _p not in sys.path and os.path.isdir(_p):
        sys.path.insert(0, _p)

import numpy as np
import ml_dtypes

import concourse.bacc as bacc
import concourse.bass as bass
import concourse.mybir as mybir
import concourse.tile as tile
from concourse.bass_utils import run_bass_kernel_spmd

R = 8          # cores
N = 8192       # nodes
P = N // R     # local rows = 1024
H = 128        # hidden
L = 4          # layers
NB = 64        # global 128-node blocks
KH = 2         # AllGather halves per layer
KL = 4         # klo blocks per half (KH*KL*128 == P)
HC = 512       # output column chunk

F32 = mybir.dt.float32
BF16 = mybir.dt.bfloat16
FP8 = mybir.dt.float8e4

LOG_A = float(np.log(2.0) / (1 << 23))
LOG_B = float(-np.log(2.0) * (127 + 0.0450466))

ADJ_FP8 = os.environ.get("K_ADJ", "fp8") == "fp8"
A2A = os.environ.get("K_COLL", "ag") == "a2a"
SP_MODE = os.environ.get("K_SP", "expln")  # expln | composed | act
SP_ACT = SP_MODE == "act"
WARM_AG = os.environ.get("K_WARM", "1") == "1"
ADT = FP8 if ADJ_FP8 else BF16

LAST_RESULT = None
_NC_CACHE = {}


def build_nc():
    nc = bacc.Bacc("TRN2", target_bir_lowering=False, debug=False, num_devices=R)

    at_ext = nc.declare_dram_parameter("atT", [128, NB, P], ADT, isOutput=False)
    featT_ext = nc.declare_dram_parameter("featT", [H, P], F32, isOutput=False)
    ws_ext = nc.declare_dram_parameter("ws", [L, H, H], BF16, isOutput=False)
    bsT_ext = nc.declare_dram_parameter("bsT", [H, L], F32, isOutput=False)
    out_ext = nc.declare_dram_parameter("out", [H, P], F32, isOutput=True)

    groups = [list(range(R))]

    with tile.TileContext(nc) as tc:
        with (
            tc.tile_pool(name="big", bufs=1) as big,
            tc.tile_pool(name="sb", bufs=1) as sb,
            tc.tile_pool(name="ftl", bufs=2) as ftl_pool,
            tc.tile_pool(name="fs", bufs=2) as fs_pool,
            tc.tile_pool(name="gst", bufs=2) as gst_pool,
            tc.tile_pool(name="gsb", bufs=2) as gsb_pool,
            tc.tile_pool(name="sp", bufs=4) as sp_pool,
            tc.tile_pool(name="psd", bufs=1, space="PSUM") as psd,
            tc.tile_pool(name="psg", bufs=1, space="PSUM") as psg,
            tc.tile_pool(name="psy", bufs=2, space="PSUM") as psy,
            tc.tile_pool(name="dram", bufs=1, space="DRAM") as dram,
        ):
            # ---- warm the collective path first (cold cost ~45us staging) ----
            if WARM_AG:
                WS_ = int(os.environ.get("K_WARMSZ", "8"))
                warm_in = dram.tile([WS_, H], BF16, name="warm_in")
                warm_out = dram.tile([R * WS_, H], BF16, addr_space="Shared", name="warm_out")
                nc.gpsimd.collective_compute(
                    "AllGather",
                    mybir.AluOpType.bypass,
                    replica_groups=groups,
                    ins=[warm_in[:, :]],
                    outs=[warm_out[:, :]],
                )

            # Pre-load the exp+ln activation table set once; with it
            # dominating every Exp/Ln below, the act-table pass inserts no
            # further (thrashing) loads. Index 6 = natural_log_exp_and_others
            # in act_info.json.
            nc.scalar.add_instruction(
                mybir.InstLoadActFuncSet(
                    name="preload_act_expln", act_func_set_id=6, ins=[], outs=[]
                )
            )

            # ---- persistent SBUF ----
            at = big.tile([128, NB, P], ADT, name="at")
            ones = sb.tile([128, 128], ADT, name="ones")
            nc.vector.memset(ones[:, :], 1.0)
            w_sb = sb.tile([128, L, H], BF16, name="w_sb")
            nc.sync.dma_start(out=w_sb[:, :, :], in_=ws_ext.rearrange("l k h -> k l h"))
            bsT_sb = sb.tile([H, L], F32, name="bsT_sb")
            nc.scalar.dma_start(out=bsT_sb[:, :], in_=bsT_ext[:, :])
            isd_rep = sb.tile([128, P], F32, name="isd_rep")

            # ---- adj^T load first: 8 octet DMAs over the two hwdge queues ----
            load_engs = [nc.sync, nc.scalar]
            for q in range(8):
                load_engs[q % 2].dma_start(
                    out=at[:, q * 8 : (q + 1) * 8, :],
                    in_=at_ext[:, q * 8 : (q + 1) * 8, :],
                )

            ftl = ftl_pool.tile([H, P], F32, name="ftl", tag="ftl")
            nc.sync.dma_start(out=ftl[:, :], in_=featT_ext[:, :])

            # ---- deg: ones-stationary matmul, replicated across partitions ----
            deg_ps = psd.tile([128, P], F32, name="deg_ps", tag="deg")
            for b in range(NB):
                for h2 in range(2):
                    nc.tensor.matmul(
                        deg_ps[:, h2 * HC : (h2 + 1) * HC],
                        ones[:, :],
                        at[:, b, h2 * HC : (h2 + 1) * HC],
                        start=(b == 0),
                        stop=(b == NB - 1),
                    )
            # isd = deg^(-1/2) = exp(-0.5*ln(deg)) — keeps the whole program
            # on the single exp+ln activation table set (no table thrash).
            nc.scalar.activation(
                isd_rep[:, :], deg_ps[:, :], mybir.ActivationFunctionType.Ln
            )
            nc.scalar.activation(
                isd_rep[:, :], isd_rep[:, :], mybir.ActivationFunctionType.Exp,
                scale=-0.5,
            )

            # ---- per-layer helpers ----
            state = {}

            def emit_g_half(l, khi):
                """pre-scale + transform + stage + AllGather for node half khi
                of layer l. Produces AG output dram tile."""
                st = state[l]
                ftl_t, fs_t, gps_t, gst_t = st["ftl"], st["fs"], st["gps"], st["gst"]
                cs = slice(khi * HC, (khi + 1) * HC)
                nc.vector.tensor_tensor(
                    fs_t[:, cs], ftl_t[:, cs], isd_rep[:, cs], mybir.AluOpType.mult
                )
                for j in range(KL):
                    nb = khi * KL + j
                    nc.tensor.matmul(
                        gps_t[:, nb, :],
                        fs_t[:, nb * 128 : (nb + 1) * 128],
                        w_sb[:, l, :],
                        start=True,
                        stop=True,
                    )
                nc.vector.tensor_copy(
                    gst_t[:, khi * KL : (khi + 1) * KL, :],
                    gps_t[:, khi * KL : (khi + 1) * KL, :],
                )
                g_in = dram.tile([HC, H], BF16, name=f"g_in_{l}_{khi}")
                (nc.sync if khi == 0 else nc.scalar).dma_start(
                    out=g_in.rearrange("(p klo) h -> p klo h", klo=KL),
                    in_=gst_t[:, khi * KL : (khi + 1) * KL, :],
                )
                g_out = dram.tile([R * HC, H], BF16, addr_space="Shared", name=f"g_out_{l}_{khi}")
                nc.gpsimd.collective_compute(
                    "AllGather",
                    mybir.AluOpType.bypass,
                    replica_groups=groups,
                    ins=[g_in[:, :]],
                    outs=[g_out[:, :]],
                )
                return g_out

            def emit_gsb_load(l, khi, g_out):
                """Load AG output into SBUF [128, khi*8+r, klo, h] layout."""
                st = state[l]
                gsb_t = st["gsb"]
                src = g_out.rearrange("(r p klo) h -> r p klo h", r=R, klo=KL)
                engs = [nc.sync, nc.scalar]
                for r in range(R):
                    engs[r % 2].dma_start(
                        out=gsb_t[:, khi * R + r, :, :], in_=src[r]
                    )

            def new_state(l, ftl_t=None):
                st = {
                    "ftl": ftl_t
                    if ftl_t is not None
                    else ftl_pool.tile([H, P], F32, name=f"ftl{l}", tag="ftl"),
                    "fs": fs_pool.tile([H, P], BF16, name=f"fs{l}", tag="fs"),
                    "gps": psg.tile([128, KH * KL, H], F32, name=f"gps{l}", tag="gps"),
                    "gst": gst_pool.tile([128, KH * KL, H], BF16, name=f"gst{l}", tag="gst"),
                    "gsb": gsb_pool.tile([128, KH * R, KL, H], BF16, name=f"gsb{l}", tag="gsb"),
                }
                state[l] = st
                return st

            def emit_epilogue(l, ch, yt):
                """softplus(yt*isd + b) -> next ftl chunk (or output)."""
                cs = slice(ch * HC, (ch + 1) * HC)
                if l < L - 1:
                    dst = state[l + 1]["ftl"]
                else:
                    dst = state["out"]
                x1 = sp_pool.tile([H, HC], F32, name="x1", tag="sp_a")
                nc.vector.tensor_tensor(
                    x1[:, :], yt[:, cs], isd_rep[:, cs], mybir.AluOpType.mult
                )
                if SP_ACT:
                    nc.scalar.activation(
                        dst[:, cs],
                        x1[:, :],
                        mybir.ActivationFunctionType.Softplus,
                        bias=bsT_sb[:, l : l + 1],
                        scale=1.0,
                    )
                elif SP_MODE == "expln":
                    # softplus(x+b) = ln(exp(x+b) + 1); Exp and Ln share one
                    # activation table set (natural_log_exp_and_others).
                    z0 = sp_pool.tile([H, HC], F32, name="z0", tag="sp_b")
                    nc.scalar.activation(
                        z0[:, :], x1[:, :], mybir.ActivationFunctionType.Exp,
                        bias=bsT_sb[:, l : l + 1], scale=1.0,
                    )
                    nc.scalar.activation(
                        dst[:, cs], z0[:, :], mybir.ActivationFunctionType.Ln,
                        bias=1.0, scale=1.0,
                    )
                else:
                    z0 = sp_pool.tile([H, HC], F32, name="z0", tag="sp_b")
                    nc.scalar.activation(
                        z0[:, :], x1[:, :], mybir.ActivationFunctionType.Exp,
                        bias=bsT_sb[:, l : l + 1], scale=1.0,
                    )
                    z = sp_pool.tile([H, HC], F32, name="z", tag="sp_c")
                    nc.vector.tensor_scalar_add(z[:, :], z0[:, :], 1.0)
                    y0 = sp_pool.tile([H, HC], F32, name="y0", tag="sp_a")
                    nc.vector.tensor_scalar(
                        y0[:, :], z[:, :].bitcast(mybir.dt.int32), LOG_A, LOG_B,
                        mybir.AluOpType.mult, mybir.AluOpType.add,
                    )
                    w_e = sp_pool.tile([H, HC], F32, name="w_e", tag="sp_b")
                    nc.scalar.activation(
                        w_e[:, :], y0[:, :], mybir.ActivationFunctionType.Exp,
                        scale=-1.0,
                    )
                    t1 = sp_pool.tile([H, HC], F32, name="t1", tag="sp_c")
                    nc.vector.tensor_tensor(
                        t1[:, :], z[:, :], w_e[:, :], mybir.AluOpType.mult
                    )
                    nc.vector.tensor_scalar_add(t1[:, :], t1[:, :], -1.0)
                    nc.vector.tensor_tensor(
                        dst[:, cs], t1[:, :], y0[:, :], mybir.AluOpType.add
                    )
                if l == L - 1:
                    nc.sync.dma_start(out=out_ext[:, cs], in_=dst[:, cs])

            # lhsT for mm slot i = khi*32 + r*4 + klo
            def slot_lhsT(st, i):
                khi, rr, klo = i // 32, (i % 32) // 4, i % 4
                return st["gsb"][:, khi * R + rr, klo, :]

            def emit_mm_wave(st, yt, ch, khi, j0=0, j1=32):
                """Slots [khi*32+j0, khi*32+j1) of column chunk ch. PSUM group
                per chunk: start on slot 0, stop on slot 63."""
                cs = slice(ch * HC, (ch + 1) * HC)
                for i in range(khi * 32 + j0, khi * 32 + j1):
                    nc.tensor.matmul(
                        yt[:, cs],
                        slot_lhsT(st, i),
                        at[:, i, cs],
                        start=(i == 0),
                        stop=(i == NB - 1),
                    )

            # ---- layer 0 g + AG in prep ----
            new_state(0, ftl_t=ftl)
            gouts = []
            for khi in range(KH):
                gouts.append(emit_g_half(0, khi))
            for khi in range(KH):
                emit_gsb_load(0, khi, gouts[khi])

            # ---- layers ----
            for l in range(L):
                st = state[l]
                if l < L - 1:
                    new_state(l + 1)
                else:
                    state["out"] = ftl_pool.tile([H, P], F32, name="ftl_out", tag="ftl")
                yt = psy.tile([H, P], F32, name=f"yt{l}", tag="yt")
                # PE order: A-khi0, B-khi0, A-khi1 (keeps PE streaming while
                # AG#1 flies), then epilogue A feeds next layer's khi0
                # transform+AG early, B-khi1 runs under that AG.
                emit_mm_wave(st, yt, 0, 0)
                emit_mm_wave(st, yt, 1, 0)
                emit_mm_wave(st, yt, 0, 1)
                emit_epilogue(l, 0, yt)
                emit_mm_wave(st, yt, 1, 1, 0, 10)
                if l < L - 1:
                    g_out0 = emit_g_half(l + 1, 0)
                emit_mm_wave(st, yt, 1, 1, 10, 32)
                emit_epilogue(l, 1, yt)
                if l < L - 1:
                    emit_gsb_load(l + 1, 0, g_out0)
                    g_out1 = emit_g_half(l + 1, 1)
                    emit_gsb_load(l + 1, 1, g_out1)

    nc.compile()
    return nc


def kernel(atom_pos, atom_emb, dist_adj, Ws, bs):
    global LAST_RESULT
    atom_pos = np.asarray(atom_pos, dtype=np.float32)
    atom_emb = np.asarray(atom_emb, dtype=np.float32)
    dist_adj = np.asarray(dist_adj, dtype=np.float32)
    Ws = np.asarray(Ws, dtype=np.float32)
    bs = np.asarray(bs, dtype=np.float32)

    feat = np.concatenate([atom_pos, atom_emb], axis=-1)  # [N, H]
    ws_bf = Ws.astype(ml_dtypes.bfloat16)
    bsT = np.ascontiguousarray(bs.T)  # [H, L]
    adj_np_dt = ml_dtypes.float8_e4m3 if ADJ_FP8 else ml_dtypes.bfloat16

    if "nc" not in _NC_CACHE:
        _NC_CACHE["nc"] = build_nc()
    nc = _NC_CACHE["nc"]

    in_maps = []
    for c in range(R):
        rows = slice(c * P, (c + 1) * P)
        # adj^T of the local row-block, tiled to SBUF layout
        # [p, khi, r, klo, c] then merged to [128, 64, 1024]:
        # slot i = khi*32 + r*4 + klo holds nodes r*1024+khi*512+klo*128+[0,128)
        blockT = dist_adj[rows].T  # [N, P] view
        at_h = (
            blockT.reshape(R, KH, KL, 128, P)
            .transpose(3, 1, 0, 2, 4)
            .reshape(128, NB, P)
            .astype(adj_np_dt)
        )
        in_maps.append(
            {
                "atT": at_h,
                "featT": np.ascontiguousarray(feat[rows].T),
                "ws": ws_bf,
                "bsT": bsT,
            }
        )

    trace = os.environ.get("K_TRACE", "0") == "1"
    kw = {}
    if trace:
        kw["trace_cores"] = list(range(R))
        kw["stitch_traces"] = os.environ.get("K_STITCH", "0") == "1"
    LAST_RESULT = run_bass_kernel_spmd(
        nc, in_maps, core_ids=list(range(R)), trace=trace, **kw
    )
    outs = [LAST_RESULT.results[c]["out"] for c in range(R)]  # each [H, P]
    return np.concatenate([o.T for o in outs], axis=0).astype(np.float32)


if __name__ == "__main__":
    rng = np.random.default_rng(0)
    out = kernel(
        rng.standard_normal((N, 3)).astype(np.float32),
        rng.standard_normal((N, 125)).astype(np.float32),
        rng.random((N, N), dtype=np.float32),
        (rng.standard_normal((L, H, H)) / np.sqrt(H)).astype(np.float32),
        np.zeros((L, H), np.float32),
    )
    print("out", out.shape, out.dtype, float(np.abs(out).mean()))


# revision 22
# speedup vs baseline: 1.2164x; 1.2164x over previous
"""AtomPosGNN distributed Trainium2 kernel (8 NeuronCores) — v2.

Reference computation (N=8192 nodes, H=128 features, L=4 layers):
    feat = concat(atom_pos, atom_emb)            # [N, 128]
    deg = dist_adj.sum(-1); isd = rsqrt(deg)
    for l in range(4):
        h = (feat * isd[:, None]) @ Ws[l]
        h = dist_adj @ h
        feat = softplus(h * isd[:, None] + bs[l])

Strategy (row shard, P=1024 rows per core):
  - Host ships adj^T for the local row-block PRE-TILED into the exact
    SBUF layout [128p, 64 slot, 1024c] (slot = (khi, r, klo) AG order)
    and cast to fp8e4 (numerically free for adj: verified 1.08e-3 final
    rel err, same as bf16). 8MB/core, one pass, 64KB/partition lines.
  - deg via PE matmul with an all-ones [128,128] fp8 stationary: output
    is deg replicated across partitions (no DRAM broadcast bounce).
    Overlaps the adj load.
  - Per layer: g = (feat*isd)@W in two node-halves (khi); each half is
    staged and AllGathered independently (2 collectives/layer) so the
    gather pipelines behind the big matmul of the previous half/chunk.
  - Big matmul y^T[h,c] = sum_n g[n,h] adjT[n,c]: g tiles stationary
    (bf16), adj^T streams (fp8), 2 column chunks of 512 for
    epilogue/AG overlap; epilogue = DVE isd-mult + ACT Softplus.
  - adj is read from HBM exactly once; layers run entirely from SBUF.
"""

import os
import sys

for _p in ("/opt/trn_rl_repo",):
    if _p not in sys.path and os.path.isdir(_p):
        sys.path.insert(0, _p)

import numpy as np
import ml_dtypes

import concourse.bacc as bacc
import concourse.bass as bass
import concourse.mybir as mybir
import concourse.tile as tile
from concourse.bass_utils import run_bass_kernel_spmd

R = 8          # cores
N = 8192       # nodes
P = N // R     # local rows = 1024
H = 128        # hidden
L = 4          # layers
NB = 64        # global 128-node blocks
KH = 2         # AllGather halves per layer
KL = 4         # klo blocks per half (KH*KL*128 == P)
HC = 512       # output column chunk

F32 = mybir.dt.float32
BF16 = mybir.dt.bfloat16
FP8 = mybir.dt.float8e4

LOG_A = float(np.log(2.0) / (1 << 23))
LOG_B = float(-np.log(2.0) * (127 + 0.0450466))

ADJ_FP8 = os.environ.get("K_ADJ", "fp8") == "fp8"
A2A = os.environ.get("K_COLL", "ag") == "a2a"
SP_MODE = os.environ.get("K_SP", "expln")  # expln | composed | act
SP_ACT = SP_MODE == "act"
WARM_AG = os.environ.get("K_WARM", "1") == "1"
ADT = FP8 if ADJ_FP8 else BF16

LAST_RESULT = None
_NC_CACHE = {}


def build_nc():
    nc = bacc.Bacc("TRN2", target_bir_lowering=False, debug=False, num_devices=R)

    at_ext = nc.declare_dram_parameter("atT", [128, NB, P], ADT, isOutput=False)
    featT_ext = nc.declare_dram_parameter("featT", [H, P], F32, isOutput=False)
    ws_ext = nc.declare_dram_parameter("ws", [L, H, H], BF16, isOutput=False)
    bsT_ext = nc.declare_dram_parameter("bsT", [H, L], F32, isOutput=False)
    out_ext = nc.declare_dram_parameter("out", [H, P], F32, isOutput=True)

    groups = [list(range(R))]

    with tile.TileContext(nc) as tc:
        with (
            tc.tile_pool(name="big", bufs=1) as big,
            tc.tile_pool(name="sb", bufs=1) as sb,
            tc.tile_pool(name="ftl", bufs=2) as ftl_pool,
            tc.tile_pool(name="fs", bufs=2) as fs_pool,
            tc.tile_pool(name="gst", bufs=2) as gst_pool,
            tc.tile_pool(name="gsb", bufs=2) as gsb_pool,
            tc.tile_pool(name="sp", bufs=4) as sp_pool,
            tc.tile_pool(name="psd", bufs=1, space="PSUM") as psd,
            tc.tile_pool(name="psg", bufs=1, space="PSUM") as psg,
            tc.tile_pool(name="psy", bufs=2, space="PSUM") as psy,
            tc.tile_pool(name="dram", bufs=1, space="DRAM") as dram,
        ):
            # ---- warm the collective path first (cold cost ~45us staging) ----
            if WARM_AG:
                WS_ = int(os.environ.get("K_WARMSZ", "8"))
                warm_in = dram.tile([WS_, H], BF16, name="warm_in")
                warm_out = dram.tile([R * WS_, H], BF16, addr_space="Shared", name="warm_out")
                nc.gpsimd.collective_compute(
                    "AllGather",
                    mybir.AluOpType.bypass,
                    replica_groups=groups,
                    ins=[warm_in[:, :]],
                    outs=[warm_out[:, :]],
                )

            # Pre-load the exp+ln activation table set once; with it
            # dominating every Exp/Ln below, the act-table pass inserts no
            # further (thrashing) loads. Index 6 = natural_log_exp_and_others
            # in act_info.json.
            nc.scalar.add_instruction(
                mybir.InstLoadActFuncSet(
                    name="preload_act_expln", act_func_set_id=6, ins=[], outs=[]
                )
            )

            # ---- persistent SBUF ----
            at = big.tile([128, NB, P], ADT, name="at")
            ones = sb.tile([128, 128], ADT, name="ones")
            nc.vector.memset(ones[:, :], 1.0)
            w_sb = sb.tile([128, L, H], BF16, name="w_sb")
            nc.sync.dma_start(out=w_sb[:, :, :], in_=ws_ext.rearrange("l k h -> k l h"))
            bsT_sb = sb.tile([H, L], F32, name="bsT_sb")
            nc.scalar.dma_start(out=bsT_sb[:, :], in_=bsT_ext[:, :])
            isd_rep = sb.tile([128, P], F32, name="isd_rep")

            # ---- adj^T load first: 8 octet DMAs over the two hwdge queues ----
            load_engs = [nc.sync, nc.scalar]
            for q in range(8):
                load_engs[q % 2].dma_start(
                    out=at[:, q * 8 : (q + 1) * 8, :],
                    in_=at_ext[:, q * 8 : (q + 1) * 8, :],
                )

            ftl = ftl_pool.tile([H, P], F32, name="ftl", tag="ftl")
            nc.sync.dma_start(out=ftl[:, :], in_=featT_ext[:, :])

            # ---- deg: ones-stationary matmul, replicated across partitions ----
            deg_ps = psd.tile([128, P], F32, name="deg_ps", tag="deg")
            for b in range(NB):
                for h2 in range(2):
                    nc.tensor.matmul(
                        deg_ps[:, h2 * HC : (h2 + 1) * HC],
                        ones[:, :],
                        at[:, b, h2 * HC : (h2 + 1) * HC],
                        start=(b == 0),
                        stop=(b == NB - 1),
                    )
            # isd = deg^(-1/2) = exp(-0.5*ln(deg)) — keeps the whole program
            # on the single exp+ln activation table set (no table thrash).
            nc.scalar.activation(
                isd_rep[:, :], deg_ps[:, :], mybir.ActivationFunctionType.Ln
            )
            nc.scalar.activation(
                isd_rep[:, :], isd_rep[:, :], mybir.ActivationFunctionType.Exp,
                scale=-0.5,
            )

            # ---- per-layer helpers ----
            state = {}

            def emit_g_half(l, khi):
                """pre-scale + transform + stage + AllGather for node half khi
                of layer l. Produces AG output dram tile."""
                st = state[l]
                ftl_t, fs_t, gps_t, gst_t = st["ftl"], st["fs"], st["gps"], st["gst"]
                cs = slice(khi * HC, (khi + 1) * HC)
                nc.vector.tensor_tensor(
                    fs_t[:, cs], ftl_t[:, cs], isd_rep[:, cs], mybir.AluOpType.mult
                )
                for j in range(KL):
                    nb = khi * KL + j
                    nc.tensor.matmul(
                        gps_t[:, nb, :],
                        fs_t[:, nb * 128 : (nb + 1) * 128],
                        w_sb[:, l, :],
                        start=True,
                        stop=True,
                    )
                nc.vector.tensor_copy(
                    gst_t[:, khi * KL : (khi + 1) * KL, :],
                    gps_t[:, khi * KL : (khi + 1) * KL, :],
                )
                g_in = dram.tile([HC, H], BF16, name=f"g_in_{l}_{khi}")
                (nc.sync if khi == 0 else nc.scalar).dma_start(
                    out=g_in.rearrange("(p klo) h -> p klo h", klo=KL),
                    in_=gst_t[:, khi * KL : (khi + 1) * KL, :],
                )
                g_out = dram.tile([R * HC, H], BF16, addr_space="Shared", name=f"g_out_{l}_{khi}")
                if A2A:
                    # AllGather emulated via AllToAll: input = 8 replicas of
                    # the local half (stride-0 leading dim), out chunk r =
                    # rank r's half — identical layout to AllGather output.
                    rep_in = bass.AP(
                        tensor=g_in.tensor,
                        offset=g_in.offset,
                        ap=[[0, R], [H, HC], [1, H]],
                    )
                    nc.gpsimd.collective_compute(
                        "AllToAll",
                        mybir.AluOpType.bypass,
                        replica_groups=groups,
                        ins=[rep_in],
                        outs=[g_out[:, :]],
                    )
                else:
                    nc.gpsimd.collective_compute(
                        "AllGather",
                        mybir.AluOpType.bypass,
                        replica_groups=groups,
                        ins=[g_in[:, :]],
                        outs=[g_out[:, :]],
                    )
                return g_out

            def emit_gsb_load(l, khi, g_out):
                """Load AG output into SBUF [128, khi*8+r, klo, h] layout."""
                st = state[l]
                gsb_t = st["gsb"]
                src = g_out.rearrange("(r p klo) h -> r p klo h", r=R, klo=KL)
                engs = [nc.sync, nc.scalar]
                for r in range(R):
                    engs[r % 2].dma_start(
                        out=gsb_t[:, khi * R + r, :, :], in_=src[r]
                    )

            def new_state(l, ftl_t=None):
                st = {
                    "ftl": ftl_t
                    if ftl_t is not None
                    else ftl_pool.tile([H, P], F32, name=f"ftl{l}", tag="ftl"),
                    "fs": fs_pool.tile([H, P], BF16, name=f"fs{l}", tag="fs"),
                    "gps": psg.tile([128, KH * KL, H], F32, name=f"gps{l}", tag="gps"),
                    "gst": gst_pool.tile([128, KH * KL, H], BF16, name=f"gst{l}", tag="gst"),
                    "gsb": gsb_pool.tile([128, KH * R, KL, H], BF16, name=f"gsb{l}", tag="gsb"),
                }
                state[l] = st
                return st

            def emit_epilogue(l, ch, yt):
                """softplus(yt*isd + b) -> next ftl chunk (or output)."""
                cs = slice(ch * HC, (ch + 1) * HC)
                if l < L - 1:
                    dst = state[l + 1]["ftl"]
                else:
                    dst = state["out"]
                x1 = sp_pool.tile([H, HC], F32, name="x1", tag="sp_a")
                nc.vector.tensor_tensor(
                    x1[:, :], yt[:, cs], isd_rep[:, cs], mybir.AluOpType.mult
                )
                if SP_ACT:
                    nc.scalar.activation(
                        dst[:, cs],
                        x1[:, :],
                        mybir.ActivationFunctionType.Softplus,
                        bias=bsT_sb[:, l : l + 1],
                        scale=1.0,
                    )
                elif SP_MODE == "expln":
                    # softplus(x+b) = ln(exp(x+b) + 1); Exp and Ln share one
                    # activation table set (natural_log_exp_and_others).
                    z0 = sp_pool.tile([H, HC], F32, name="z0", tag="sp_b")
                    nc.scalar.activation(
                        z0[:, :], x1[:, :], mybir.ActivationFunctionType.Exp,
                        bias=bsT_sb[:, l : l + 1], scale=1.0,
                    )
                    nc.scalar.activation(
                        dst[:, cs], z0[:, :], mybir.ActivationFunctionType.Ln,
                        bias=1.0, scale=1.0,
                    )
                else:
                    z0 = sp_pool.tile([H, HC], F32, name="z0", tag="sp_b")
                    nc.scalar.activation(
                        z0[:, :], x1[:, :], mybir.ActivationFunctionType.Exp,
                        bias=bsT_sb[:, l : l + 1], scale=1.0,
                    )
                    z = sp_pool.tile([H, HC], F32, name="z", tag="sp_c")
                    nc.vector.tensor_scalar_add(z[:, :], z0[:, :], 1.0)
                    y0 = sp_pool.tile([H, HC], F32, name="y0", tag="sp_a")
                    nc.vector.tensor_scalar(
                        y0[:, :], z[:, :].bitcast(mybir.dt.int32), LOG_A, LOG_B,
                        mybir.AluOpType.mult, mybir.AluOpType.add,
                    )
                    w_e = sp_pool.tile([H, HC], F32, name="w_e", tag="sp_b")
                    nc.scalar.activation(
                        w_e[:, :], y0[:, :], mybir.ActivationFunctionType.Exp,
                        scale=-1.0,
                    )
                    t1 = sp_pool.tile([H, HC], F32, name="t1", tag="sp_c")
                    nc.vector.tensor_tensor(
                        t1[:, :], z[:, :], w_e[:, :], mybir.AluOpType.mult
                    )
                    nc.vector.tensor_scalar_add(t1[:, :], t1[:, :], -1.0)
                    nc.vector.tensor_tensor(
                        dst[:, cs], t1[:, :], y0[:, :], mybir.AluOpType.add
                    )
                if l == L - 1:
                    nc.sync.dma_start(out=out_ext[:, cs], in_=dst[:, cs])

            # lhsT for mm slot i = khi*32 + r*4 + klo
            def slot_lhsT(st, i):
                khi, rr, klo = i // 32, (i % 32) // 4, i % 4
                return st["gsb"][:, khi * R + rr, klo, :]

            def emit_mm_wave(st, yt, ch, khi, j0=0, j1=32):
                """Slots [khi*32+j0, khi*32+j1) of column chunk ch. PSUM group
                per chunk: start on slot 0, stop on slot 63."""
                cs = slice(ch * HC, (ch + 1) * HC)
                for i in range(khi * 32 + j0, khi * 32 + j1):
                    nc.tensor.matmul(
                        yt[:, cs],
                        slot_lhsT(st, i),
                        at[:, i, cs],
                        start=(i == 0),
                        stop=(i == NB - 1),
                    )

            # ---- layer 0 g + AG in prep ----
            new_state(0, ftl_t=ftl)
            gouts = []
            for khi in range(KH):
                gouts.append(emit_g_half(0, khi))
            for khi in range(KH):
                emit_gsb_load(0, khi, gouts[khi])

            # ---- layers ----
            for l in range(L):
                st = state[l]
                if l < L - 1:
                    new_state(l + 1)
                else:
                    state["out"] = ftl_pool.tile([H, P], F32, name="ftl_out", tag="ftl")
                yt = psy.tile([H, P], F32, name=f"yt{l}", tag="yt")
                # PE order: A-khi0, B-khi0, A-khi1 (keeps PE streaming while
                # AG#1 flies), then epilogue A feeds next layer's khi0
                # transform+AG early, B-khi1 runs under that AG.
                emit_mm_wave(st, yt, 0, 0)
                emit_mm_wave(st, yt, 1, 0)
                emit_mm_wave(st, yt, 0, 1)
                emit_epilogue(l, 0, yt)
                emit_mm_wave(st, yt, 1, 1, 0, 10)
                if l < L - 1:
                    g_out0 = emit_g_half(l + 1, 0)
                emit_mm_wave(st, yt, 1, 1, 10, 32)
                emit_epilogue(l, 1, yt)
                if l < L - 1:
                    emit_gsb_load(l + 1, 0, g_out0)
                    g_out1 = emit_g_half(l + 1, 1)
                    emit_gsb_load(l + 1, 1, g_out1)

    nc.compile()
    return nc


def kernel(atom_pos, atom_emb, dist_adj, Ws, bs):
    global LAST_RESULT
    atom_pos = np.asarray(atom_pos, dtype=np.float32)
    atom_emb = np.asarray(atom_emb, dtype=np.float32)
    dist_adj = np.asarray(dist_adj, dtype=np.float32)
    Ws = np.asarray(Ws, dtype=np.float32)
    bs = np.asarray(bs, dtype=np.float32)

    feat = np.concatenate([atom_pos, atom_emb], axis=-1)  # [N, H]
    ws_bf = Ws.astype(ml_dtypes.bfloat16)
    bsT = np.ascontiguousarray(bs.T)  # [H, L]
    adj_np_dt = ml_dtypes.float8_e4m3 if ADJ_FP8 else ml_dtypes.bfloat16

    if "nc" not in _NC_CACHE:
        _NC_CACHE["nc"] = build_nc()
    nc = _NC_CACHE["nc"]

    in_maps = []
    for c in range(R):
        rows = slice(c * P, (c + 1) * P)
        # adj^T of the local row-block, tiled to SBUF layout
        # [p, khi, r, klo, c] then merged to [128, 64, 1024]:
        # slot i = khi*32 + r*4 + klo holds nodes r*1024+khi*512+klo*128+[0,128)
        blockT = dist_adj[rows].T  # [N, P] view
        at_h = (
            blockT.reshape(R, KH, KL, 128, P)
            .transpose(3, 1, 0, 2, 4)
            .reshape(128, NB, P)
            .astype(adj_np_dt)
        )
        in_maps.append(
            {
                "atT": at_h,
                "featT": np.ascontiguousarray(feat[rows].T),
                "ws": ws_bf,
                "bsT": bsT,
            }
        )

    trace = os.environ.get("K_TRACE", "0") == "1"
    kw = {}
    if trace:
        kw["trace_cores"] = list(range(R))
        kw["stitch_traces"] = os.environ.get("K_STITCH", "0") == "1"
    LAST_RESULT = run_bass_kernel_spmd(
        nc, in_maps, core_ids=list(range(R)), trace=trace, **kw
    )
    outs = [LAST_RESULT.results[c]["out"] for c in range(R)]  # each [H, P]
    return np.concatenate([o.T for o in outs], axis=0).astype(np.float32)


if __name__ == "__main__":
    rng = np.random.default_rng(0)
    out = kernel(
        rng.standard_normal((N, 3)).astype(np.float32),
        rng.standard_normal((N, 125)).astype(np.float32),
        rng.random((N, N), dtype=np.float32),
        (rng.standard_normal((L, H, H)) / np.sqrt(H)).astype(np.float32),
        np.zeros((L, H), np.float32),
    )
    print("out", out.shape, out.dtype, float(np.abs(out).mean()))
